# revision 17
# baseline (speedup 1.0000x reference)
"""Trainium2 Bass kernel: span bag-of-words embedding (nn_BOW_24781961298234).

Math: out[b,s,:] = sum over UNIQUE word ids u in span [i,j) of W[u,:] + bias.
Scatter-free reformulation (multi-hot "set" semantics via prev-occurrence):
    E[t,:]    = W[word_encs[b,t], :]                      (bulk dma_gather)
    mask[t,s] = [i_s <= t] - [j_s <= t]                   (= [i<=t<j])
    out[b,s]  = sum_t mask[t,s] * E[t]
              - sum_{dup tokens t} [i_s <= prev_t][t < j_s] * E[t]  + bias
where prev[b,t] = last t'<t with word_encs[b,t']==word_encs[b,t] (-1 if none).
Dup tokens (prev>=0, ~2.6 per 512-token row) are compacted into one extra
128-slot "dup chunk"; the bias is folded in as one more dup slot whose
gathered row is -b and whose mask column is -1 everywhere.

Gather: ONE logical gather of 17x128 rows per core via gpsimd.dma_gather
(SWDGE bulk gather: ~1us desc-gen per call vs ~1us per 128 rows for
indirect_dma_start).  dma_gather indices are int16, so the host first
permutes the vocab table: the <=16384 ids actually used by the batch are
moved to the front of W_perm (full [V+1, D] table stays resident in HBM;
host does only index bookkeeping + one table permutation).  HW probes:
  - dma_gather crashes the device above 1024 indices per call -> split
    into 3 calls (1024/1024/128).
  - matmul with PSUM partition offset != 0 crashes the device -> per-batch
    [64, D] psum tiles; dup matmuls contract the full 128-partition dup
    chunk with host-zeroed (sentinel 60000) cross-batch mask blocks.
Everything 16-bit where possible: W_perm/E/mask in fp16 (DVE 2x perf mode,
half the gather HBM traffic; exact for 0/1 masks and integer span bounds
<= 2048; output accumulates in f32 PSUM).

Sharding: data-parallel over batch; 32 batches / 8 cores = 4 per core.
"""

import numpy as np

B, S, T, V, D = 32, 64, 512, 50257, 128
NCORES = 8
BPC = B // NCORES     # batches per core
NC = T // 128         # 128-token chunks per sequence
NCHUNK = 1 + BPC * NC           # dup chunk + 16 main chunks
NIDX = NCHUNK * 128             # 2176
KDUP = 32                       # dup slots per batch (slot 127 = bias)
BIG = 60000.0                   # +/- sentinel, fp16-safe

# aux tile column layout (fp16 columns)
C_IJ = 0                  # ij_all   [128, 2*BPC*S]  (i block | j block)
C_ID = C_IJ + 2 * BPC * S         # iD_all   [128, BPC*S]
C_JD = C_ID + BPC * S             # jD_all   [128, BPC*S]
C_TC = C_JD + BPC * S             # t_cols   [128, NC]
C_F32 = C_TC + NC                 # f32 scalars (2 fp16 slots each):
                                  #   prev_col, tdup_col, t_col[0..3]
C_IDX = C_F32 + 12                # identity row idx 0..127, int16 wrap
NAUX = C_IDX + 8

_cache = {}


def _build_nc():
    import concourse.tile as tile
    from concourse import bacc, mybir

    f32, f16 = mybir.dt.float32, mybir.dt.float16
    i16 = mybir.dt.int16
    Alu = mybir.AluOpType

    nc = bacc.Bacc("TRN2", target_bir_lowering=False, debug=False,
                   num_devices=NCORES, num_swdge_queues=2)

    w_d = nc.dram_tensor("wp", [V + 1, D], f16, kind="ExternalInput")
    idx_d = nc.dram_tensor("idx", [128, NIDX // 16], i16, kind="ExternalInput")
    aux_d = nc.dram_tensor("aux", [128, NAUX], f16, kind="ExternalInput")
    out_d = nc.dram_tensor("out", [D, BPC * S], f32, kind="ExternalOutput")

    with tile.TileContext(nc) as tc:
        with (
            tc.tile_pool(name="sb", bufs=1) as sb,
            tc.tile_pool(name="ps", bufs=1, space="PSUM") as ps,
        ):
            # idx first: gathers are the critical path
            idx_t = sb.tile([128, NIDX // 16], i16)
            nc.sync.dma_start(idx_t[:], idx_d[:])
            aux_t = sb.tile([128, NAUX], f16)
            nc.sync.dma_start(aux_t[:], aux_d[:])
            scal_f32 = aux_t[:, C_F32:C_F32 + 12].bitcast(f32)

            # bulk gather, split into <=1024-idx calls (HW ucode limit)
            E = sb.tile([128, NCHUNK * D], f16)
            splits = [(0, 8), (8, 16), (16, NCHUNK)]
            for c0, c1 in splits:
                n = (c1 - c0) * 128
                nc.gpsimd.dma_gather(
                    E[:, c0 * D:c1 * D].rearrange("p (c d) -> p c d",
                                                  c=c1 - c0),
                    w_d[:],
                    idx_t[:, c0 * 8:c1 * 8],
                    n,
                    n,
                    D,
                )

            out_s = sb.tile([128, BPC * S], f32)
            ij = aux_t[:, C_IJ:C_IJ + 2 * BPC * S]

            # main mask: per-chunk tensor_scalar keeps every non-scalar
            # operand packed-fp16 so the DVE 2x perf mode applies (a
            # broadcast t operand would force 1x); t scalar APs must be f32
            # C_all[p, c, x] = [ij[x] <= t_c(p)];  M[p, c, bs] = C_i - C_j
            t_cols = aux_t[:, C_TC:C_TC + NC]
            C_all = sb.tile([128, NC * 2 * BPC * S], f16)
            c4 = C_all[:].rearrange("p (c x) -> p c x", c=NC)
            ij_bc = ij[:, None, :].to_broadcast([128, NC, 2 * BPC * S])
            t_bc = t_cols[:, :, None].to_broadcast([128, NC, 2 * BPC * S])
            nc.vector.tensor_tensor(out=c4, in0=ij_bc, in1=t_bc, op=Alu.is_le)

            M = sb.tile([128, NC * BPC * S], f16)
            m3 = M[:].rearrange("p (c y) -> p c y", c=NC)
            ci = c4[:, :, 0:BPC * S]
            cj = c4[:, :, BPC * S:2 * BPC * S]
            nc.vector.tensor_tensor(out=m3, in0=ci, in1=cj, op=Alu.subtract)

            # dup-chunk mask: Cd[p, bs] = -[iD <= prev_p] * [jD > tdup_p]
            Ad = sb.tile([128, BPC * S], f16)
            Bd = sb.tile([128, BPC * S], f16)
            Cd = sb.tile([128, BPC * S], f16)
            nc.vector.tensor_scalar(
                out=Ad[:], in0=aux_t[:, C_ID:C_ID + BPC * S],
                scalar1=scal_f32[:, 0:1], scalar2=-1.0,
                op0=Alu.is_le, op1=Alu.mult)
            nc.vector.tensor_scalar(
                out=Bd[:], in0=aux_t[:, C_JD:C_JD + BPC * S],
                scalar1=scal_f32[:, 1:2], scalar2=None,
                op0=Alu.is_gt)
            nc.vector.tensor_tensor(out=Cd[:], in0=Ad[:], in1=Bd[:],
                                    op=Alu.mult)

            # per-batch psum accumulation, transposed (out[d, s]): matmul
            # time ~ out free size, so rhs = 64-wide mask halves PE time
            for b in range(BPC):
                out_ps = ps.tile([128, S], f32, tag=f"psum{b}")
                po = out_ps[:]
                # dup matmul first so the group's stop matmul is the one
                # waiting on the last-arriving gather chunk
                nc.tensor.matmul(
                    out=po, lhsT=E[:, 0:D],
                    rhs=Cd[:, b * S:(b + 1) * S], start=True, stop=False)
                for c in range(NC):
                    ch = 1 + b * NC + c
                    nc.tensor.matmul(
                        out=po,
                        lhsT=E[:, ch * D:(ch + 1) * D],
                        rhs=M[:, (c * BPC + b) * S:(c * BPC + b) * S + S],
                        start=False, stop=(c == NC - 1))
                nc.vector.tensor_copy(out_s[:, b * S:(b + 1) * S], po)
            # SWDGE-prepped output stores: a row-identity scatter-add into
            # the (pre-zeroed) output buffer is a plain store of out_s.
            # Desc-gen runs on the idle Pool engine during the matmuls (the
            # out_s read is deferred to the trigger); each trigger then
            # fires its store as a cheap sequencer op, skipping the ~1.3us
            # HWDGE+DGE store latency of a sync-engine dma_start. Two
            # stores on separate SWDGE queues: batches 0-2 fire while batch
            # 3 finishes; the final store is then a single tiny transfer.
            dma_sem0 = nc.alloc_semaphore("out_dma0")
            nc.gpsimd.dma_scatter_add(
                out_d[:, 0:(BPC - 1) * S],
                out_s[:, 0:(BPC - 1) * S].rearrange("p (c e) -> p c e", c=1),
                aux_t[:, C_IDX:C_IDX + 8].bitcast(i16),
                128,
                128,
                (BPC - 1) * S,
                elem_step=BPC * S,
                prepare_only=True,
                sem=dma_sem0,
                queue_num=0,
            )
            dma_sem1 = nc.alloc_semaphore("out_dma1")
            nc.gpsimd.dma_scatter_add(
                out_d[:, (BPC - 1) * S:BPC * S],
                out_s[:, (BPC - 1) * S:BPC * S]
                .rearrange("p (c e) -> p c e", c=1),
                aux_t[:, C_IDX:C_IDX + 8].bitcast(i16),
                128,
                128,
                S,
                elem_step=BPC * S,
                prepare_only=True,
                sem=dma_sem1,
                queue_num=1,
            )
            nc.gpsimd.trigger_dma(count=None, queue_num=0)
            nc.gpsimd.trigger_dma(count=None, queue_num=1)

    nc.compile()
    return nc


def get_nc():
    if "nc" not in _cache:
        _cache["nc"] = _build_nc()
    return _cache["nc"]


def _compute_prev(word_encs):
    """prev[b,t] = last t'<t with the same word id, else -1."""
    prev = np.full(word_encs.shape, -1, np.int64)
    for b in range(word_encs.shape[0]):
        last = {}
        row = word_encs[b]
        for t in range(word_encs.shape[1]):
            v = int(row[t])
            p = last.get(v, -1)
            prev[b, t] = p
            last[v] = t
    return prev


def make_in_maps(word_encs, span_idxs, W, b):
    word_encs = np.asarray(word_encs)
    span_idxs = np.asarray(span_idxs)
    W = np.asarray(W)
    bias = np.asarray(b, dtype=np.float32)

    # vocab compaction: ids used by this batch -> [0, n_used) (int16-safe)
    used = np.unique(word_encs)
    n_used = len(used)
    assert n_used + 1 <= 32767, n_used
    lut = np.zeros(V, np.int32)
    lut[used] = np.arange(n_used)
    w_perm = np.zeros((V + 1, D), np.float16)
    w_perm[:n_used] = W[used].astype(np.float16)
    w_perm[n_used] = (-bias).astype(np.float16)

    ids_new = lut[word_encs].astype(np.int16)     # [B, T]
    prev = _compute_prev(word_encs)

    in_maps = []
    for m in range(NCORES):
        bsl = slice(m * BPC, (m + 1) * BPC)
        rows = ids_new[bsl]                        # [BPC, T]
        sp = span_idxs[bsl].astype(np.float32)     # [BPC, S, 2]
        pv = prev[bsl]                             # [BPC, T]

        idx_list = np.zeros(NIDX, np.int16)
        idx_list[128:] = rows.reshape(BPC * NC * 128)

        aux = np.zeros((128, NAUX), np.float16)
        i_bs = sp[:, :, 0].reshape(BPC * S)
        j_bs = sp[:, :, 1].reshape(BPC * S)
        aux[:, C_IJ:C_IJ + BPC * S] = i_bs[None, :]
        aux[:, C_IJ + BPC * S:C_IJ + 2 * BPC * S] = j_bs[None, :]
        aux[:, C_TC:C_TC + NC] = (
            np.arange(128, dtype=np.float32)[:, None]
            + 128.0 * np.arange(NC, dtype=np.float32)[None, :])

        iD = np.full((128, BPC * S), BIG, np.float32)   # default: never fire
        jD = np.zeros((128, BPC * S), np.float32)
        prev_col = np.full(128, -5.0, np.float32)
        tdup_col = np.zeros(128, np.float32)

        for bb in range(BPC):
            dup_ts = np.nonzero(pv[bb] >= 0)[0]
            limit = KDUP - 1 if bb == BPC - 1 else KDUP
            assert len(dup_ts) <= limit, (
                f"batch row {m * BPC + bb}: {len(dup_ts)} dup tokens "
                f"exceed {limit} slots")
            for k, t in enumerate(dup_ts):
                p = bb * KDUP + k
                idx_list[p] = rows[bb, t]
                prev_col[p] = pv[bb, t]
                tdup_col[p] = t
                iD[p, bb * S:(bb + 1) * S] = i_bs[bb * S:(bb + 1) * S]
                jD[p, bb * S:(bb + 1) * S] = j_bs[bb * S:(bb + 1) * S]

        # bias slot: fires (-1) for every (b, s); gathered row is -bias
        idx_list[127] = n_used
        iD[127, :] = -BIG
        jD[127, :] = BIG
        prev_col[127] = 0.0
        tdup_col[127] = 0.0

        aux[:, C_ID:C_ID + BPC * S] = iD
        aux[:, C_JD:C_JD + BPC * S] = jD
        tcol_f32 = (np.arange(128, dtype=np.float32)[:, None]
                    + 128.0 * np.arange(NC, dtype=np.float32)[None, :])
        aux[:, C_F32:C_F32 + 12] = np.concatenate(
            [np.stack([prev_col, tdup_col], axis=1).astype(np.float32),
             tcol_f32], axis=1).view(np.float16)
        ridx = np.tile(np.arange(128, dtype=np.int16).reshape(8, 16).T,
                       (8, 1))
        aux[:, C_IDX:C_IDX + 8] = ridx.view(np.float16)

        # idx wrap: element k at [k%16, k//16], tiled to 128 partitions
        idx_in = np.ascontiguousarray(
            np.tile(idx_list.reshape(NIDX // 16, 16).T, (8, 1)))

        in_maps.append({"wp": w_perm, "idx": idx_in, "aux": aux})
    return in_maps


def kernel(word_encs, span_idxs, W, b):
    from concourse.bass_utils import run_bass_kernel_spmd

    nc = get_nc()
    in_maps = make_in_maps(word_encs, span_idxs, W, b)
    res = run_bass_kernel_spmd(nc, in_maps, core_ids=list(range(NCORES)))
    out = np.concatenate(
        [r["out"].reshape(D, BPC, S).transpose(1, 2, 0)
         for r in res.results], axis=0)
    return np.ascontiguousarray(out, dtype=np.float32)
